# revision 5
# baseline (speedup 1.0000x reference)
"""MoE (top-2, SwiGLU, 8 experts) Trainium2 kernel, expert-parallel over 8 cores.

Contract: kernel(**inputs) takes the FULL unsharded inputs
  x [2, 1024, 1024] f32, router_w [1024, 8] f32,
  w1 [8, 1024, 2048] f32, w2 [8, 2048, 1024] f32, w3 [8, 1024, 2048] f32,
  top_k = 2
and returns the FULL output [2, 1024, 1024] f32.

Strategy (expert-parallel, dense gating):
  - Core e holds expert e's weights (cast to bf16 on host as part of shard
    marshaling) and the full token set.
  - On device: fp32 router (logits -> softmax -> top-2 mask) for all tokens,
    per-token gate g = prob_e if prob_e is among the row's top-2 else 0.
  - Input pre-scaling (reference scales before the expert MLP): xs = g * x.
    For unselected tokens g = 0 so the SwiGLU output is exactly 0; summing
    the 8 per-core partial outputs on host therefore equals the reference
    scatter-add combine.
  - MLP in bf16 with fp32 PSUM accumulation; layer 2 emits token-major
    output directly by using h^T tiles as the stationary matmul operand.
"""

import numpy as np
import ml_dtypes

import concourse.bass as bass
import concourse.bacc as bacc
import concourse.mybir as mybir
import concourse.tile as tile
from concourse.bass import ts
from concourse.bass_utils import run_bass_kernel_spmd

P = 128
T = 2048  # tokens
D = 1024  # model dim
H = 2048  # hidden dim
E = 8     # experts == cores
KD = D // P    # 8 contraction chunks for dim
KH = H // P    # 16 contraction chunks for hidden
TT = T // P    # 16 token tiles
NC = 512       # matmul moving-dim chunk (one PSUM bank fp32)
TQ = 512       # token quarter processed per L1/L2 round
NQ = T // TQ   # 4 quarters

F32 = mybir.dt.float32
BF16 = mybir.dt.bfloat16
AX = mybir.AxisListType
ALU = mybir.AluOpType
ACTF = mybir.ActivationFunctionType


def build_moe_nc():
    nc = bacc.Bacc("TRN2", target_bir_lowering=False, debug=False)

    xT_h = nc.dram_tensor("xT", [D, T], F32, kind="ExternalInput").ap()
    rw_h = nc.dram_tensor("rw", [D, E], F32, kind="ExternalInput").ap()
    esel_h = nc.dram_tensor("esel", [P, E], F32, kind="ExternalInput").ap()
    w1_h = nc.dram_tensor("w1", [D, H], BF16, kind="ExternalInput").ap()
    w3_h = nc.dram_tensor("w3", [D, H], BF16, kind="ExternalInput").ap()
    w2_h = nc.dram_tensor("w2", [H, D], BF16, kind="ExternalInput").ap()
    y_h = nc.dram_tensor("y", [T, D], F32, kind="ExternalOutput").ap()
    g_dram = nc.dram_tensor("g_dram", [TT, P, 1], F32).ap()

    with tile.TileContext(nc) as tc:
        with tc.tile_pool(name="wA", bufs=1) as wA:
            # L1 weights + scaled activations, resident through L1
            w1sb = wA.tile([P, KD, H], BF16, tag="w1")
            w3sb = wA.tile([P, KD, H], BF16, tag="w3")
            for k in range(KD):
                nc.sync.dma_start(
                    w1sb[:, k, :], w1_h[ts(k, P), :]
                )
                nc.sync.dma_start(
                    w3sb[:, k, :], w3_h[ts(k, P), :]
                )
            xsT = wA.tile([P, KD, T], BF16, tag="xsT")

            with tc.tile_pool(name="xf", bufs=1) as xf:
                xTsb = xf.tile([P, KD, T], F32, tag="xT")
                for k in range(KD):
                    nc.sync.dma_start(
                        xTsb[:, k, :], xT_h[ts(k, P), :]
                    )
                rwsb = xf.tile([P, KD, E], F32, tag="rw")
                nc.sync.dma_start(rwsb, rw_h.rearrange("(k p) e -> p k e", p=P))
                eselsb = xf.tile([P, E], F32, tag="esel")
                nc.sync.dma_start(eselsb, esel_h)
                Gsb = xf.tile([P, T], F32, tag="G")

                with (
                    tc.tile_pool(name="rps", bufs=2, space="PSUM") as rps,
                    tc.tile_pool(name="rsb", bufs=3) as rsb,
                ):
                    for i in range(TT):
                        lg = rps.tile([P, E], F32, tag="lg")
                        for k in range(KD):
                            nc.tensor.matmul(
                                lg,
                                lhsT=xTsb[:, k, ts(i, P)],
                                rhs=rwsb[:, k, :],
                                start=(k == 0),
                                stop=(k == KD - 1),
                            )
                        negm = rsb.tile([P, 1], F32, tag="negm")
                        nc.vector.tensor_reduce(
                            negm, lg, axis=AX.X, op=ALU.max, negate=True
                        )
                        probs = rsb.tile([P, E], F32, tag="probs")
                        ssum = rsb.tile([P, 1], F32, tag="ssum")
                        nc.scalar.activation(
                            probs, lg, ACTF.Exp, bias=negm, scale=1.0, accum_out=ssum
                        )
                        rcp = rsb.tile([P, 1], F32, tag="rcp")
                        nc.vector.reciprocal(rcp, ssum)
                        nc.vector.tensor_scalar_mul(probs, probs, rcp)
                        m1 = rsb.tile([P, 1], F32, tag="m1")
                        nc.vector.tensor_reduce(m1, probs, axis=AX.X, op=ALU.max)
                        mask = rsb.tile([P, E], F32, tag="mask")
                        nc.vector.tensor_scalar(mask, probs, m1, None, op0=ALU.is_ge)
                        masked = rsb.tile([P, E], F32, tag="masked")
                        nc.vector.scalar_tensor_tensor(
                            masked, mask, -1e30, probs, op0=ALU.mult, op1=ALU.add
                        )
                        m2 = rsb.tile([P, 1], F32, tag="m2")
                        nc.vector.tensor_reduce(m2, masked, axis=AX.X, op=ALU.max)
                        pse = rsb.tile([P, E], F32, tag="pse")
                        nc.vector.tensor_tensor(pse, probs, eselsb, op=ALU.mult)
                        pex = rsb.tile([P, 1], F32, tag="pex")
                        nc.vector.tensor_reduce(pex, pse, axis=AX.X, op=ALU.add)
                        ge = rsb.tile([P, 1], F32, tag="ge")
                        nc.vector.tensor_tensor(ge, pex, m2, op=ALU.is_ge)
                        gate = rsb.tile([P, 1], F32, tag="gate")
                        nc.vector.tensor_tensor(gate, pex, ge, op=ALU.mult)
                        nc.sync.dma_start(g_dram[i], gate)

                # broadcast gates across partitions, one DMA per token quarter
                for q in range(NQ):
                    gb_ap = bass.AP(
                        tensor=g_dram.tensor, offset=q * TQ, ap=[[0, P], [1, TQ]]
                    )
                    nc.gpsimd.dma_start(Gsb[:, ts(q, TQ)], gb_ap)
                # xs^T = x^T * G, cast to bf16
                for q in range(NQ):
                    for k in range(KD):
                        nc.vector.tensor_tensor(
                            xsT[:, k, ts(q, TQ)],
                            xTsb[:, k, ts(q, TQ)],
                            Gsb[:, ts(q, TQ)],
                            op=ALU.mult,
                        )

            # xf freed here; MLP pools open
            with (
                tc.tile_pool(name="wB", bufs=1) as wB,
                tc.tile_pool(name="hp", bufs=2) as hp,
                tc.tile_pool(name="mp", bufs=3) as mp,
                tc.tile_pool(name="pp", bufs=2, space="PSUM") as pp,
            ):
                w2sb = wB.tile([P, KH, D], BF16, tag="w2")
                for k in range(KH):
                    nc.sync.dma_start(
                        w2sb[:, k, :], w2_h[ts(k, P), :]
                    )
                for q in range(NQ):
                    hT = hp.tile([P, KH, TQ], BF16, tag="hT")
                    # ---- layer 1: h^T = silu(w1^T xs^T) * (w3^T xs^T) ----
                    for m in range(KH):
                        ps1 = pp.tile([P, NC], F32, tag="ps1")
                        ps3 = pp.tile([P, NC], F32, tag="ps3")
                        for k in range(KD):
                            nc.tensor.matmul(
                                ps1,
                                lhsT=w1sb[:, k, ts(m, P)],
                                rhs=xsT[:, k, ts(q, TQ)],
                                start=(k == 0),
                                stop=(k == KD - 1),
                            )
                        for k in range(KD):
                            nc.tensor.matmul(
                                ps3,
                                lhsT=w3sb[:, k, ts(m, P)],
                                rhs=xsT[:, k, ts(q, TQ)],
                                start=(k == 0),
                                stop=(k == KD - 1),
                            )
                        sg = mp.tile([P, NC], BF16, tag="sg")
                        nc.scalar.activation(sg, ps1, ACTF.Sigmoid)
                        u13 = mp.tile([P, NC], BF16, tag="u13")
                        # u = ps1 * ps3 (one PSUM operand per op: ps1 via sg path)
                        nc.vector.tensor_tensor(u13, sg, ps1, op=ALU.mult)
                        nc.vector.tensor_tensor(
                            hT[:, m, :], u13, ps3, op=ALU.mult
                        )
                    # ---- layer 2: y = (h^T)^T w2, token-major output ----
                    for i in range(TQ // P):
                        for n2 in range(D // NC):
                            psy = pp.tile([P, NC], F32, tag="psy")
                            for k in range(KH):
                                nc.tensor.matmul(
                                    psy,
                                    lhsT=hT[:, k, ts(i, P)],
                                    rhs=w2sb[:, k, ts(n2, NC)],
                                    start=(k == 0),
                                    stop=(k == KH - 1),
                                )
                            ysb = mp.tile([P, NC], F32, tag="ysb")
                            nc.scalar.activation(ysb, psy, ACTF.Copy)
                            nc.sync.dma_start(
                                y_h[ts(q * (TQ // P) + i, P), ts(n2, NC)], ysb
                            )
    nc.compile()
    return nc


_NC_CACHE = None


def _get_nc():
    global _NC_CACHE
    if _NC_CACHE is None:
        _NC_CACHE = build_moe_nc()
    return _NC_CACHE


def make_in_maps(x, router_w, w1, w2, w3):
    xt = np.ascontiguousarray(np.asarray(x, np.float32).reshape(T, D))
    xT = np.ascontiguousarray(xt.T)
    rw = np.ascontiguousarray(np.asarray(router_w, np.float32))
    w1b = np.asarray(w1).astype(ml_dtypes.bfloat16)
    w2b = np.asarray(w2).astype(ml_dtypes.bfloat16)
    w3b = np.asarray(w3).astype(ml_dtypes.bfloat16)
    in_maps = []
    for e in range(E):
        esel = np.zeros((P, E), np.float32)
        esel[:, e] = 1.0
        in_maps.append(
            {
                "xT": xT,
                "rw": rw,
                "esel": esel,
                "w1": np.ascontiguousarray(w1b[e]),
                "w3": np.ascontiguousarray(w3b[e]),
                "w2": np.ascontiguousarray(w2b[e]),
            }
        )
    return in_maps


def kernel(x, router_w, w1, w2, w3, top_k):
    assert int(top_k) == 2
    nc = _get_nc()
    in_maps = make_in_maps(x, router_w, w1, w2, w3)
    res = run_bass_kernel_spmd(nc, in_maps, list(range(E))).results
    out = res[0]["y"].astype(np.float32)
    for e in range(1, E):
        out = out + res[e]["y"]
    return out.reshape(2, T // 2, D)
